# revision 9
# baseline (speedup 1.0000x reference)
"""TRN2 Bass kernel for nn_CustomMLPLayer_10806137716759.

Computes, for x [1, S, F] f32 and W [D, F] f32:
  true_value [1, S, D] = x @ W.T
  neuron_mask [F] bool = counts > floor(mean(counts) * 0.95),
    counts[f] = #{s : x[0, s, f] > 0}

Strategy: tensor-parallel over D across 8 NeuronCores (each core owns a
512-column shard of W and all of x).  The contraction axis F (padded
11008 -> 11264) is split into NQ chunks streamed through SBUF; partial
[s, d] outputs accumulate in PSUM within a chunk and in an SBUF partials
buffer across chunks.  Matmuls run in float32r (TF32-like, 1 cycle/row
at N=512).  SPLIT=1 rounds operands during a casting DMA (f32->f32r,
~7e-5 rel err); SPLIT=3 splits both operands hi/lo on DVE and runs
3 matmul passes (~2e-7 rel err, near-fp32).

Per-neuron activation counts: each core receives its k-chunks in a
rotated order (matmul accumulation is order-independent) and computes
counts only on its first STATS_H chunks; across 8 cores every chunk is
covered.  The tiny mean/cutoff/mask finalization happens on host during
the gather step.
"""
import sys
if '/opt/trn_rl_repo' not in sys.path:
    sys.path.insert(0, '/opt/trn_rl_repo')

import numpy as np

S, F, D = 2048, 11008, 4096
NCORES = 8
DSH = D // NCORES        # 512 output columns per core
KT = 88                  # padded k tiles (F padded to 11264)
FP = KT * 128
NS = S // 128            # 16 s-strips
SPLIT = 3                # 1 = single f32r pass, 3 = hi/lo split
NQ = 4 if SPLIT == 1 else 11     # k chunks streamed through SBUF
KQ = KT // NQ
STATS_H = -(-NQ // NCORES)       # leading chunks with on-device stats

_CACHE = {}


def _chunk_order(c):
    """Global k-chunk processing order for core c (stats coverage)."""
    return [(c * STATS_H + h) % NQ for h in range(NQ)]


def _build_program(n_iters=1):
    """Build the per-core Bass program.  n_iters>1 wraps the body in a
    hardware For_i loop (used only for timing measurements)."""
    import contextlib
    import concourse.tile as tile
    from concourse import mybir, bacc

    f32 = mybir.dt.float32
    f32r = mybir.dt.float32r
    AX = mybir.AxisListType
    OP = mybir.AluOpType

    nc = bacc.Bacc("TRN2", target_bir_lowering=False, debug=False)
    xs = nc.dram_tensor("xs", [NQ, NS, 128, KQ, 128], f32, kind="ExternalInput")
    wt = nc.dram_tensor("wt", [NQ, 128, KQ, DSH], f32, kind="ExternalInput")
    out = nc.dram_tensor("out", [NS, 128, DSH], f32, kind="ExternalOutput")
    cnt = nc.dram_tensor("cnt", [128, STATS_H * KQ], f32, kind="ExternalOutput")

    with tile.TileContext(nc) as tc:
        loop_cm = (tc.For_i(0, n_iters, 1) if n_iters > 1
                   else contextlib.nullcontext())
        with loop_cm, \
             tc.tile_pool(name="wp", bufs=2) as wp, \
             tc.tile_pool(name="whp", bufs=2) as whp, \
             tc.tile_pool(name="wlp", bufs=2) as wlp, \
             tc.tile_pool(name="xp", bufs=3) as xp, \
             tc.tile_pool(name="xhp", bufs=3) as xhp, \
             tc.tile_pool(name="xlp", bufs=3) as xlp, \
             tc.tile_pool(name="pt", bufs=1) as pt, \
             tc.tile_pool(name="op_", bufs=2) as op_, \
             tc.tile_pool(name="scp", bufs=2) as scp, \
             tc.tile_pool(name="rdp", bufs=2) as rdp, \
             tc.tile_pool(name="ps", bufs=4, space="PSUM") as ps:

            partials = pt.tile([128, NS, DSH], f32)
            counts = pt.tile([128, STATS_H * KQ], f32)
            nc.vector.memset(counts[:], 0.0)

            for h in range(NQ):
                if SPLIT == 1:
                    w_hi = whp.tile([128, KQ, DSH], f32r)
                    nc.gpsimd.dma_start(out=w_hi[:], in_=wt[h])  # casting DMA
                    w_ops = [w_hi]
                else:
                    w_raw = wp.tile([128, KQ, DSH], f32)
                    nc.sync.dma_start(out=w_raw[:], in_=wt[h])
                    w_hi = whp.tile([128, KQ, DSH], f32r)
                    nc.vector.tensor_copy(w_hi[:], w_raw[:])
                    w_lo = wlp.tile([128, KQ, DSH], f32r)
                    nc.vector.tensor_sub(w_lo[:], w_raw[:],
                                         w_hi[:].bitcast(f32))
                    w_ops = [w_hi, w_lo]

                for n in range(NS):
                    if SPLIT == 1:
                        x_hi = xhp.tile([128, KQ, 128], f32r)
                        nc.gpsimd.dma_start(out=x_hi[:], in_=xs[h, n])
                        pairs = [(x_hi, w_hi)]
                    else:
                        x_raw = xp.tile([128, KQ, 128], f32)
                        nc.sync.dma_start(out=x_raw[:], in_=xs[h, n])
                        x_hi = xhp.tile([128, KQ, 128], f32r)
                        nc.vector.tensor_copy(x_hi[:], x_raw[:])
                        x_lo = xlp.tile([128, KQ, 128], f32r)
                        nc.vector.tensor_sub(x_lo[:], x_raw[:],
                                             x_hi[:].bitcast(f32))
                        # hi*hi + hi*lo + lo*hi  (lo*lo dropped)
                        pairs = [(x_hi, w_ops[0]), (x_hi, w_ops[1]),
                                 (x_lo, w_ops[0])]

                    psum = ps.tile([128, DSH], f32)
                    n_mm = len(pairs) * KQ
                    i = 0
                    for j2 in range(KQ):
                        for (xt, wtile) in pairs:
                            nc.tensor.matmul(
                                psum[:],
                                xt[:, j2, :].bitcast(f32r),
                                wtile[:, j2, :].bitcast(f32r),
                                start=(i == 0), stop=(i == n_mm - 1))
                            i += 1

                    if h == 0:
                        nc.vector.tensor_copy(partials[:, n, :], psum[:])
                    elif h < NQ - 1:
                        nc.vector.tensor_add(partials[:, n, :],
                                             partials[:, n, :], psum[:])
                    else:
                        ot = op_.tile([128, DSH], f32)
                        nc.vector.tensor_add(ot[:], partials[:, n, :], psum[:])
                        nc.sync.dma_start(out=out[n], in_=ot[:])

                    if h < STATS_H:
                        # per-neuron activation counts for this chunk's k range
                        sc = scp.tile([128, KQ, 128], f32)
                        nc.vector.tensor_scalar(sc[:], x_hi[:].bitcast(f32),
                                                0.0, None, OP.is_gt)
                        rd = rdp.tile([128, KQ], f32)
                        nc.vector.tensor_reduce(rd[:], sc[:], axis=AX.X,
                                                op=OP.add)
                        nc.vector.tensor_add(counts[:, h * KQ:(h + 1) * KQ],
                                             counts[:, h * KQ:(h + 1) * KQ],
                                             rd[:])

            nc.sync.dma_start(out=cnt[:], in_=counts[:])

    nc.compile()
    return nc


def _prep_inputs(x, W):
    """Host-side relayout: both operands contraction-major, pre-tiled,
    k-chunks rotated per core for distributed stats coverage."""
    x2 = np.ascontiguousarray(x.reshape(S, F), dtype=np.float32)
    xpad = np.zeros((S, FP), dtype=np.float32)
    xpad[:, :F] = x2
    # [n, j, g, j2, p] -> [g, n, p, j2, j]
    xs_g = np.ascontiguousarray(
        xpad.reshape(NS, 128, NQ, KQ, 128).transpose(2, 0, 4, 3, 1))

    Wp = np.zeros((D, FP), dtype=np.float32)
    Wp[:, :F] = W

    xss, wts = [], []
    for c in range(NCORES):
        order = _chunk_order(c)
        xss.append(np.ascontiguousarray(xs_g[order]))
        Wc = Wp[c * DSH:(c + 1) * DSH]          # [DSH, FP]
        # [d, g, j2, p] -> [g, p, j2, d]
        wt_g = Wc.reshape(DSH, NQ, KQ, 128).transpose(1, 3, 2, 0)
        wts.append(np.ascontiguousarray(wt_g[order]))
    return xss, wts


def _run(nc, xss, wts, trace=False):
    from concourse.bass_utils import run_bass_kernel_spmd
    in_maps = [{"xs": xss[c], "wt": wts[c]} for c in range(NCORES)]
    return run_bass_kernel_spmd(nc, in_maps, core_ids=list(range(NCORES)),
                                trace=trace)


def _assemble(res):
    outs = [res.results[c]["out"].reshape(S, DSH) for c in range(NCORES)]
    true_value = np.concatenate(outs, axis=1).reshape(1, S, D)

    # counts: core c's cnt column block h corresponds to global chunk
    # (c*STATS_H + h) % NQ
    counts_g = np.zeros((NQ, 128, KQ), dtype=np.float32)
    seen = set()
    for c in range(NCORES):
        cntm = res.results[c]["cnt"]             # [128, STATS_H*KQ]
        for h in range(STATS_H):
            g = (c * STATS_H + h) % NQ
            if g in seen:
                continue
            seen.add(g)
            counts_g[g] = cntm[:, h * KQ:(h + 1) * KQ]
    # counts_g[g, p, j2] -> f = (g*KQ + j2)*128 + p
    counts = counts_g.transpose(0, 2, 1).reshape(FP)[:F]
    mean = counts.astype(np.float32).mean(dtype=np.float32)
    cutoff = np.floor(mean * np.float32(0.95))
    neuron_mask = counts > cutoff
    return true_value, neuron_mask


def kernel(x, W):
    if "nc" not in _CACHE:
        _CACHE["nc"] = _build_program()
    xss, wts = _prep_inputs(np.asarray(x), np.asarray(W))
    res = _run(_CACHE["nc"], xss, wts)
    return _assemble(res)


# revision 10
# speedup vs baseline: 2.8183x; 2.8183x over previous
"""TRN2 Bass kernel for nn_CustomMLPLayer_10806137716759.

Computes, for x [1, S, F] f32 and W [D, F] f32:
  true_value [1, S, D] = x @ W.T
  neuron_mask [F] bool = counts > floor(mean(counts) * 0.95),
    counts[f] = #{s : x[0, s, f] > 0}

Strategy: tensor-parallel over D across 8 NeuronCores (each core owns a
512-column shard of W and all of x).  The contraction axis F (padded
11008 -> 11264) is split into NQ chunks streamed through SBUF; partial
[s, d] outputs accumulate in PSUM within a chunk and in an SBUF partials
buffer across chunks.  Matmuls run in float32r (TF32-like, 1 cycle/row
at N=512).  SPLIT=1 rounds operands during a casting DMA (f32->f32r,
~7e-5 rel err); SPLIT=3 splits both operands hi/lo on DVE and runs
3 matmul passes (~2e-7 rel err, near-fp32).

Per-neuron activation counts: each core receives its k-chunks in a
rotated order (matmul accumulation is order-independent) and computes
counts only on its first STATS_H chunks; across 8 cores every chunk is
covered.  The tiny mean/cutoff/mask finalization happens on host during
the gather step.
"""
import sys
if '/opt/trn_rl_repo' not in sys.path:
    sys.path.insert(0, '/opt/trn_rl_repo')

import numpy as np

S, F, D = 2048, 11008, 4096
NCORES = 8
DSH = D // NCORES        # 512 output columns per core
KT = 88                  # padded k tiles (F padded to 11264)
FP = KT * 128
NS = S // 128            # 16 s-strips
SPLIT = 1                # 1 = single f32r pass, 3 = hi/lo split
NQ = 4 if SPLIT == 1 else 11     # k chunks streamed through SBUF
KQ = KT // NQ
STATS_H = -(-NQ // NCORES)       # leading chunks with on-device stats

_CACHE = {}


def _chunk_order(c):
    """Global k-chunk processing order for core c (stats coverage)."""
    return [(c * STATS_H + h) % NQ for h in range(NQ)]


def _build_program(n_iters=1):
    """Build the per-core Bass program.  n_iters>1 wraps the body in a
    hardware For_i loop (used only for timing measurements)."""
    import contextlib
    import concourse.tile as tile
    from concourse import mybir, bacc

    f32 = mybir.dt.float32
    f32r = mybir.dt.float32r
    AX = mybir.AxisListType
    OP = mybir.AluOpType

    nc = bacc.Bacc("TRN2", target_bir_lowering=False, debug=False)
    xs = nc.dram_tensor("xs", [NQ, NS, 128, KQ, 128], f32, kind="ExternalInput")
    wt = nc.dram_tensor("wt", [NQ, 128, KQ, DSH], f32, kind="ExternalInput")
    out = nc.dram_tensor("out", [NS, 128, DSH], f32, kind="ExternalOutput")
    cnt = nc.dram_tensor("cnt", [128, STATS_H * KQ], f32, kind="ExternalOutput")

    with tile.TileContext(nc) as tc:
        loop_cm = (tc.For_i(0, n_iters, 1) if n_iters > 1
                   else contextlib.nullcontext())
        with loop_cm, \
             tc.tile_pool(name="wp", bufs=2) as wp, \
             tc.tile_pool(name="whp", bufs=2) as whp, \
             tc.tile_pool(name="wlp", bufs=2) as wlp, \
             tc.tile_pool(name="xp", bufs=3) as xp, \
             tc.tile_pool(name="xhp", bufs=3) as xhp, \
             tc.tile_pool(name="xlp", bufs=3) as xlp, \
             tc.tile_pool(name="pt", bufs=1) as pt, \
             tc.tile_pool(name="op_", bufs=2) as op_, \
             tc.tile_pool(name="scp", bufs=2) as scp, \
             tc.tile_pool(name="rdp", bufs=2) as rdp, \
             tc.tile_pool(name="ps", bufs=4, space="PSUM") as ps:

            partials = pt.tile([128, NS, DSH], f32)
            counts = pt.tile([128, STATS_H * KQ], f32)
            nc.vector.memset(counts[:], 0.0)

            for h in range(NQ):
                if SPLIT == 1:
                    w_hi = whp.tile([128, KQ, DSH], f32r)
                    nc.gpsimd.dma_start(out=w_hi[:], in_=wt[h])  # casting DMA
                    w_ops = [w_hi]
                else:
                    w_raw = wp.tile([128, KQ, DSH], f32)
                    nc.sync.dma_start(out=w_raw[:], in_=wt[h])
                    w_hi = whp.tile([128, KQ, DSH], f32r)
                    nc.vector.tensor_copy(w_hi[:], w_raw[:])
                    w_lo = wlp.tile([128, KQ, DSH], f32r)
                    nc.vector.tensor_sub(w_lo[:], w_raw[:],
                                         w_hi[:].bitcast(f32))
                    w_ops = [w_hi, w_lo]

                for n in range(NS):
                    if SPLIT == 1:
                        x_hi = xhp.tile([128, KQ, 128], f32r)
                        nc.gpsimd.dma_start(out=x_hi[:], in_=xs[h, n])
                        pairs = [(x_hi, w_hi)]
                    else:
                        x_raw = xp.tile([128, KQ, 128], f32)
                        nc.sync.dma_start(out=x_raw[:], in_=xs[h, n])
                        x_hi = xhp.tile([128, KQ, 128], f32r)
                        nc.vector.tensor_copy(x_hi[:], x_raw[:])
                        x_lo = xlp.tile([128, KQ, 128], f32r)
                        nc.vector.tensor_sub(x_lo[:], x_raw[:],
                                             x_hi[:].bitcast(f32))
                        # hi*hi + hi*lo + lo*hi  (lo*lo dropped)
                        pairs = [(x_hi, w_ops[0]), (x_hi, w_ops[1]),
                                 (x_lo, w_ops[0])]

                    psum = ps.tile([128, DSH], f32)
                    n_mm = len(pairs) * KQ
                    i = 0
                    for j2 in range(KQ):
                        for (xt, wtile) in pairs:
                            nc.tensor.matmul(
                                psum[:],
                                xt[:, j2, :].bitcast(f32r),
                                wtile[:, j2, :].bitcast(f32r),
                                start=(i == 0), stop=(i == n_mm - 1))
                            i += 1

                    if h == 0:
                        nc.vector.tensor_copy(partials[:, n, :], psum[:])
                    elif h < NQ - 1:
                        nc.vector.tensor_add(partials[:, n, :],
                                             partials[:, n, :], psum[:])
                    else:
                        ot = op_.tile([128, DSH], f32)
                        nc.vector.tensor_add(ot[:], partials[:, n, :], psum[:])
                        nc.sync.dma_start(out=out[n], in_=ot[:])

                    if h < STATS_H:
                        # per-neuron activation counts for this chunk's k range
                        sc = scp.tile([128, KQ, 128], f32)
                        nc.vector.tensor_scalar(sc[:], x_hi[:].bitcast(f32),
                                                0.0, None, OP.is_gt)
                        rd = rdp.tile([128, KQ], f32)
                        nc.vector.tensor_reduce(rd[:], sc[:], axis=AX.X,
                                                op=OP.add)
                        nc.vector.tensor_add(counts[:, h * KQ:(h + 1) * KQ],
                                             counts[:, h * KQ:(h + 1) * KQ],
                                             rd[:])

            nc.sync.dma_start(out=cnt[:], in_=counts[:])

    nc.compile()
    return nc


def _prep_inputs(x, W):
    """Host-side relayout: both operands contraction-major, pre-tiled,
    k-chunks rotated per core for distributed stats coverage."""
    x2 = np.ascontiguousarray(x.reshape(S, F), dtype=np.float32)
    xpad = np.zeros((S, FP), dtype=np.float32)
    xpad[:, :F] = x2
    # [n, j, g, j2, p] -> [g, n, p, j2, j]
    xs_g = np.ascontiguousarray(
        xpad.reshape(NS, 128, NQ, KQ, 128).transpose(2, 0, 4, 3, 1))

    Wp = np.zeros((D, FP), dtype=np.float32)
    Wp[:, :F] = W

    xss, wts = [], []
    for c in range(NCORES):
        order = _chunk_order(c)
        xss.append(np.ascontiguousarray(xs_g[order]))
        Wc = Wp[c * DSH:(c + 1) * DSH]          # [DSH, FP]
        # [d, g, j2, p] -> [g, p, j2, d]
        wt_g = Wc.reshape(DSH, NQ, KQ, 128).transpose(1, 3, 2, 0)
        wts.append(np.ascontiguousarray(wt_g[order]))
    return xss, wts


def _run(nc, xss, wts, trace=False):
    from concourse.bass_utils import run_bass_kernel_spmd
    in_maps = [{"xs": xss[c], "wt": wts[c]} for c in range(NCORES)]
    return run_bass_kernel_spmd(nc, in_maps, core_ids=list(range(NCORES)),
                                trace=trace)


def _assemble(res):
    outs = [res.results[c]["out"].reshape(S, DSH) for c in range(NCORES)]
    true_value = np.concatenate(outs, axis=1).reshape(1, S, D)

    # counts: core c's cnt column block h corresponds to global chunk
    # (c*STATS_H + h) % NQ
    counts_g = np.zeros((NQ, 128, KQ), dtype=np.float32)
    seen = set()
    for c in range(NCORES):
        cntm = res.results[c]["cnt"]             # [128, STATS_H*KQ]
        for h in range(STATS_H):
            g = (c * STATS_H + h) % NQ
            if g in seen:
                continue
            seen.add(g)
            counts_g[g] = cntm[:, h * KQ:(h + 1) * KQ]
    # counts_g[g, p, j2] -> f = (g*KQ + j2)*128 + p
    counts = counts_g.transpose(0, 2, 1).reshape(FP)[:F]
    mean = counts.astype(np.float32).mean(dtype=np.float32)
    cutoff = np.floor(mean * np.float32(0.95))
    neuron_mask = counts > cutoff
    return true_value, neuron_mask


def kernel(x, W):
    if "nc" not in _CACHE:
        _CACHE["nc"] = _build_program()
    xss, wts = _prep_inputs(np.asarray(x), np.asarray(W))
    res = _run(_CACHE["nc"], xss, wts)
    return _assemble(res)
